# revision 5
# baseline (speedup 1.0000x reference)
"""Trainium2 Bass kernel for nn_MoEFusion (multi-modal MoE fusion MLP).

Data-parallel across 8 NeuronCores: batch dim (32768) sharded into 8
slices of 4096, all weights (<1 MB) replicated. No collectives.

On-device dataflow (per core, feature-major "T" layout everywhere):
  featT [768, 4096] (host-pre-transposed) --DMA(cast f32r)--> SBUF
  projT[m] = proj_w[m].T @ featT[m]            (PE, f32r, accum K=768)
  xT = concat_m(projT + proj_b)                (ACT bias-add PSUM->SBUF)
  gateT = exp(gate_w.T @ xT + gate_b)          (PE + ACT Exp)
  gwT = gateT / colsum(gateT)                  (GPSIMD partition reduce, DVE)
  hT[e] = relu(W1[e].T @ xT + b1[e])           (PE + ACT Relu)
  shT[e] = hT[e] * bcast(gwT[e])               (GPSIMD bcast + DVE mul)
  fusedT = exp_b2.T @ gwT + sum_e W2[e].T @ shT[e]   (single PSUM accum)
  penT = relu(pre_w.T @ fusedT + pre_b)        (PE + ACT)
  outT = head_w.T @ penT + head_b              (PE + ACT) --DMA--> [2, 4096]
Host re-transposes/concats to [32768, 2].

float32r: PE streams fp32 data at full rate (1 col/cycle for N>=256);
walrus requires every f32r matmul operand to be *produced* as f32r, so
DMA loads cast f32->f32r (SWDGE) and ACT/DVE producers write f32r tiles.
"""

import sys

if "/opt/trn_rl_repo" not in sys.path:
    sys.path.insert(0, "/opt/trn_rl_repo")

from contextlib import ExitStack

import numpy as np

# ---- problem constants (hardcoded per contract) ----
B = 32768
NCORES = 8
BL = B // NCORES  # 4096 per core
STRIPE = 512
NM = 3
NE = 8
D_IN = 768
KIN = D_IN // 128  # 6
D_P = 128
D_X = 384
KX = D_X // 128  # 3

# ---- packed matmul-weight SBUF layout (columns of [128, WMCOLS], f32r) ----
OFF_PROJ = 0                           # [p, m*768 + k*128 + o] = proj_w[m, k*128+p, o]
OFF_W1 = OFF_PROJ + NM * KIN * 128     # 2304: [p, e*384 + k*128 + h] = exp_w1[e, k*128+p, h]
OFF_W2 = OFF_W1 + NE * KX * 128        # 5376: [p, e*128 + o] = exp_w2[e, p, o]
OFF_GATE = OFF_W2 + NE * 128           # 6400: [p, k*8 + e] = gate_w[k*128+p, e]
OFF_PRE = OFF_GATE + KX * NE           # 6424: [p, o] = pre_w[p, o]
OFF_HEAD = OFF_PRE + 64                # 6488: [p<64, c] = head_w[p, c]
OFF_B2 = OFF_HEAD + 2                  # 6490: [p<8, o] = exp_b2[p, o]
WMCOLS = OFF_B2 + 128                  # 6618

# ---- bias SBUF layout (columns of [128, WBCOLS], f32) ----
OFF_PROJB = 0                          # [p, m] = proj_b[m, p]
OFF_B1 = OFF_PROJB + NM                # [p, e] = exp_b1[e, p]
OFF_GATEB = OFF_B1 + NE                # [p<8, 0] = gate_b[p]
OFF_PREB = OFF_GATEB + 1               # [p<64, 0] = pre_b[p]
OFF_HEADB = OFF_PREB + 1               # [p<2, 0] = head_b[p]
WBCOLS = OFF_HEADB + 1                 # 14


def pack_weights(inp):
    wm = np.zeros((128, WMCOLS), np.float32)
    pw = np.asarray(inp["proj_w"], np.float32)      # [3, 768, 128]
    wm[:, OFF_PROJ:OFF_W1] = (
        pw.reshape(NM, KIN, 128, 128).transpose(2, 0, 1, 3).reshape(128, -1)
    )
    w1 = np.asarray(inp["exp_w1"], np.float32)      # [8, 384, 128]
    wm[:, OFF_W1:OFF_W2] = (
        w1.reshape(NE, KX, 128, 128).transpose(2, 0, 1, 3).reshape(128, -1)
    )
    w2 = np.asarray(inp["exp_w2"], np.float32)      # [8, 128, 128]
    wm[:, OFF_W2:OFF_GATE] = w2.transpose(1, 0, 2).reshape(128, -1)
    gw = np.asarray(inp["gate_w"], np.float32)      # [384, 8]
    wm[:, OFF_GATE:OFF_PRE] = (
        gw.reshape(KX, 128, NE).transpose(1, 0, 2).reshape(128, -1)
    )
    wm[:, OFF_PRE:OFF_HEAD] = np.asarray(inp["pre_w"], np.float32)
    wm[:64, OFF_HEAD:OFF_B2] = np.asarray(inp["head_w"], np.float32)
    wm[:8, OFF_B2:OFF_B2 + 128] = np.asarray(inp["exp_b2"], np.float32)

    wb = np.zeros((128, WBCOLS), np.float32)
    wb[:, OFF_PROJB:OFF_B1] = np.asarray(inp["proj_b"], np.float32).T
    wb[:, OFF_B1:OFF_GATEB] = np.asarray(inp["exp_b1"], np.float32).T
    wb[:8, OFF_GATEB] = np.asarray(inp["gate_b"], np.float32)
    wb[:64, OFF_PREB] = np.asarray(inp["pre_b"], np.float32)
    wb[:2, OFF_HEADB] = np.asarray(inp["head_b"], np.float32)
    return wm, wb


def build_program(n_stripes=BL // STRIPE):
    """Build the per-core Bass program (identical on all cores)."""
    import concourse.bacc as bacc
    import concourse.bass as bass
    import concourse.bass_isa as bass_isa
    import concourse.mybir as mybir
    import concourse.tile as tile

    f32 = mybir.dt.float32
    f32r = mybir.dt.float32r
    AF = mybir.ActivationFunctionType
    bl = n_stripes * STRIPE

    nc = bacc.Bacc(
        "TRN2",
        target_bir_lowering=False,
        debug=False,
        enable_asserts=False,
    )

    featT = nc.dram_tensor("featT", [NM, D_IN, bl], f32, kind="ExternalInput").ap()
    wmat = nc.dram_tensor("wmat", [128, WMCOLS], f32, kind="ExternalInput").ap()
    wbias = nc.dram_tensor("wbias", [128, WBCOLS], f32, kind="ExternalInput").ap()
    outT = nc.dram_tensor("outT", [2, bl], f32, kind="ExternalOutput").ap()

    with tile.TileContext(nc) as tc, ExitStack() as ctx:
        wp_pool = ctx.enter_context(tc.tile_pool(name="wp", bufs=1))
        feat_pool = ctx.enter_context(tc.tile_pool(name="feat", bufs=6))
        x_pool = ctx.enter_context(tc.tile_pool(name="x", bufs=6))
        gw_pool = ctx.enter_context(tc.tile_pool(name="gw", bufs=2))
        gb_pool = ctx.enter_context(tc.tile_pool(name="gb", bufs=4))
        h_pool = ctx.enter_context(tc.tile_pool(name="h", bufs=3))
        sh_pool = ctx.enter_context(tc.tile_pool(name="sh", bufs=3))
        f_pool = ctx.enter_context(tc.tile_pool(name="f", bufs=2))
        pen_pool = ctx.enter_context(tc.tile_pool(name="pen", bufs=2))
        o_pool = ctx.enter_context(tc.tile_pool(name="o", bufs=2))

        px_pool = ctx.enter_context(tc.tile_pool(name="px", bufs=2, space="PSUM"))
        ph_pool = ctx.enter_context(tc.tile_pool(name="ph", bufs=2, space="PSUM"))
        pf_pool = ctx.enter_context(tc.tile_pool(name="pf", bufs=2, space="PSUM"))
        ps_pool = ctx.enter_context(tc.tile_pool(name="ps", bufs=2, space="PSUM"))

        # preload packed weights once (cast to f32r) + biases (f32)
        W = wp_pool.tile([128, WMCOLS], f32r)
        nc.gpsimd.dma_start(W[:], wmat[:])
        Bz = wp_pool.tile([128, WBCOLS], f32)
        nc.sync.dma_start(Bz[:], wbias[:])

        def wslice(off, n, parts=128):
            return W[:parts, off : off + n]

        def bslice(off, parts=128):
            return Bz[:parts, off : off + 1]

        featT_t = featT.rearrange("m (k p) b -> m p k b", p=128)

        for s in range(n_stripes):
            bsl = slice(s * STRIPE, (s + 1) * STRIPE)

            # ---- load features, cast f32 -> f32r (1.5 MB per modality) ----
            ft = []
            for m in range(NM):
                t = feat_pool.tile([128, KIN, STRIPE], f32r, tag="feat")
                nc.gpsimd.dma_start(t[:], featT_t[m, :, :, bsl])
                ft.append(t)

            # ---- per-modality projection -> xT chunks ----
            xT = []
            for m in range(NM):
                px = px_pool.tile([128, STRIPE], f32, tag="px")
                for k in range(KIN):
                    nc.tensor.matmul(
                        px[:],
                        wslice(OFF_PROJ + m * KIN * 128 + k * 128, 128),
                        ft[m][:, k, :],
                        start=(k == 0),
                        stop=(k == KIN - 1),
                    )
                xt = x_pool.tile([128, STRIPE], f32r, tag="x")
                nc.scalar.activation(
                    xt[:], px[:], AF.Identity,
                    bias=bslice(OFF_PROJB + m), scale=1.0,
                )
                xT.append(xt)

            # ---- gate: softmax over 8 experts (partition axis) ----
            pg = ps_pool.tile([8, STRIPE], f32, tag="ps")
            for k in range(KX):
                nc.tensor.matmul(
                    pg[:],
                    wslice(OFF_GATE + k * NE, NE),
                    xT[k][:],
                    start=(k == 0),
                    stop=(k == KX - 1),
                )
            eT = gw_pool.tile([8, STRIPE], f32, tag="eT")
            nc.scalar.activation(
                eT[:], pg[:], AF.Exp, bias=bslice(OFF_GATEB, parts=8), scale=1.0
            )
            sT = gw_pool.tile([8, STRIPE], f32, tag="sT")
            nc.gpsimd.partition_all_reduce(
                sT[:], eT[:], channels=8, reduce_op=bass_isa.ReduceOp.add
            )
            rT = gw_pool.tile([8, STRIPE], f32, tag="rT")
            nc.vector.reciprocal(rT[:], sT[:])
            gwT = gw_pool.tile([8, STRIPE], f32r, tag="gwT")
            nc.vector.tensor_mul(gwT[:], eT[:], rT[:])

            # ---- experts: h = relu(W1.T x + b1); sh = h * gw[e] ----
            sh = []
            for e in range(NE):
                ph = ph_pool.tile([128, STRIPE], f32, tag="ph")
                for k in range(KX):
                    nc.tensor.matmul(
                        ph[:],
                        wslice(OFF_W1 + e * KX * 128 + k * 128, 128),
                        xT[k][:],
                        start=(k == 0),
                        stop=(k == KX - 1),
                    )
                h = h_pool.tile([128, STRIPE], f32r, tag="h")
                nc.scalar.activation(
                    h[:], ph[:], AF.Relu, bias=bslice(OFF_B1 + e), scale=1.0
                )
                grow = gw_pool.tile([1, STRIPE], f32r, tag="grow")
                nc.sync.dma_start(grow[:], gwT[e : e + 1, :])
                gb = gb_pool.tile([128, STRIPE], f32r, tag="gb")
                nc.gpsimd.partition_broadcast(gb[:], grow[:], channels=128)
                sht = sh_pool.tile([128, STRIPE], f32r, tag="sh")
                nc.vector.tensor_mul(sht[:], h[:], gb[:])
                sh.append(sht)

            # ---- fused = exp_b2.T @ gwT + sum_e W2[e].T @ sh[e] ----
            pf = pf_pool.tile([128, STRIPE], f32, tag="pf")
            nc.tensor.matmul(
                pf[:], wslice(OFF_B2, 128, parts=8), gwT[:],
                start=True, stop=False,
            )
            for e in range(NE):
                nc.tensor.matmul(
                    pf[:],
                    wslice(OFF_W2 + e * 128, 128),
                    sh[e][:],
                    start=False,
                    stop=(e == NE - 1),
                )
            fT = f_pool.tile([128, STRIPE], f32r, tag="f")
            nc.scalar.copy(fT[:], pf[:])

            # ---- penult = relu(pre_w.T @ fused + pre_b) ----
            pp = ps_pool.tile([64, STRIPE], f32, tag="ps")
            nc.tensor.matmul(pp[:], wslice(OFF_PRE, 64), fT[:],
                             start=True, stop=True)
            pen = pen_pool.tile([64, STRIPE], f32r, tag="pen")
            nc.scalar.activation(
                pen[:], pp[:], AF.Relu, bias=bslice(OFF_PREB, parts=64), scale=1.0
            )

            # ---- logits = head_w.T @ penult + head_b ----
            po = ps_pool.tile([2, STRIPE], f32, tag="ps")
            nc.tensor.matmul(po[:], wslice(OFF_HEAD, 2, parts=64), pen[:],
                             start=True, stop=True)
            ot = o_pool.tile([2, STRIPE], f32, tag="o")
            nc.scalar.activation(
                ot[:], po[:], AF.Identity, bias=bslice(OFF_HEADB, parts=2),
                scale=1.0,
            )
            nc.sync.dma_start(outT[:, bsl], ot[:])

    nc.compile()
    return nc


_PROGRAM = None


def _get_program():
    global _PROGRAM
    if _PROGRAM is None:
        _PROGRAM = build_program()
    return _PROGRAM


def make_in_maps(inputs):
    """Host-side shard + layout prep: list of 8 per-core input maps."""
    wm, wb = pack_weights(inputs)
    feats = [
        np.asarray(inputs["feat_text"], np.float32),
        np.asarray(inputs["feat_audio"], np.float32),
        np.asarray(inputs["feat_video"], np.float32),
    ]
    in_maps = []
    for c in range(NCORES):
        sl = slice(c * BL, (c + 1) * BL)
        featT = np.stack([np.ascontiguousarray(f[sl].T) for f in feats])
        in_maps.append({"featT": featT, "wmat": wm, "wbias": wb})
    return in_maps


def run_on_hw(inputs, trace=False):
    from concourse.bass_utils import run_bass_kernel_spmd

    nc = _get_program()
    in_maps = make_in_maps(inputs)
    res = run_bass_kernel_spmd(
        nc, in_maps, core_ids=list(range(NCORES)), trace=trace
    )
    out = np.concatenate([r["outT"].T for r in res.results], axis=0)
    return out, res


def kernel(**inputs):
    out, _ = run_on_hw(inputs, trace=False)
    return out


# revision 15
# speedup vs baseline: 1.0003x; 1.0003x over previous
"""Trainium2 Bass kernel for nn_MoEFusion (multi-modal MoE fusion MLP).

Data-parallel across 8 NeuronCores: batch dim (32768) sharded into 8
slices of 4096, all weights (<1 MB) replicated. No collectives.

On-device dataflow (per core, feature-major "T" layout everywhere):
  featT [768, 4096] (host-pre-transposed) --DMA(cast f32r)--> SBUF
  projT[m] = proj_w[m].T @ featT[m]            (PE, f32r, accum K=768)
  xT = concat_m(projT + proj_b)                (ACT bias-add PSUM->SBUF)
  gateT = exp(gate_w.T @ xT + gate_b)          (PE + ACT Exp)
  gwT = gateT / colsum(gateT)                  (GPSIMD partition reduce, DVE)
  hT[e] = relu(W1[e].T @ xT + b1[e])           (PE + ACT Relu)
  shT[e] = hT[e] * bcast(gwT[e])               (GPSIMD bcast + DVE mul)
  fusedT = exp_b2.T @ gwT + sum_e W2[e].T @ shT[e]   (single PSUM accum)
  penT = relu(pre_w.T @ fusedT + pre_b)        (PE + ACT)
  outT = head_w.T @ penT + head_b              (PE + ACT) --DMA--> [2, 4096]
Host re-transposes/concats to [32768, 2].

float32r: PE streams fp32 data at full rate (1 col/cycle for N>=256);
walrus requires every f32r matmul operand to be *produced* as f32r, so
DMA loads cast f32->f32r (SWDGE) and ACT/DVE producers write f32r tiles.
"""

import sys

if "/opt/trn_rl_repo" not in sys.path:
    sys.path.insert(0, "/opt/trn_rl_repo")

from contextlib import ExitStack

import numpy as np

# ---- problem constants (hardcoded per contract) ----
B = 32768
NCORES = 8
BL = B // NCORES  # 4096 per core
STRIPE = 512
NM = 3
NE = 8
D_IN = 768
KIN = D_IN // 128  # 6
D_P = 128
D_X = 384
KX = D_X // 128  # 3

# ---- packed matmul-weight SBUF layout (columns of [128, WMCOLS], f32r) ----
OFF_PROJ = 0                           # [p, m*768 + k*128 + o] = proj_w[m, k*128+p, o]
OFF_W1 = OFF_PROJ + NM * KIN * 128     # 2304: [p, e*384 + k*128 + h] = exp_w1[e, k*128+p, h]
OFF_W2 = OFF_W1 + NE * KX * 128        # 5376: [p, e*128 + o] = exp_w2[e, p, o]
OFF_GATE = OFF_W2 + NE * 128           # 6400: [p, k*8 + e] = gate_w[k*128+p, e]
OFF_PRE = OFF_GATE + KX * NE           # 6424: [p, o] = pre_w[p, o]
OFF_HEAD = OFF_PRE + 64                # 6488: [p<64, c] = head_w[p, c]
OFF_B2 = OFF_HEAD + 2                  # 6490: [p<8, o] = exp_b2[p, o]
WMCOLS = OFF_B2 + 128                  # 6618

# ---- bias SBUF layout (columns of [128, WBCOLS], f32) ----
OFF_PROJB = 0                          # [p, m] = proj_b[m, p]
OFF_B1 = OFF_PROJB + NM                # [p, e] = exp_b1[e, p]
OFF_GATEB = OFF_B1 + NE                # [p<8, 0] = gate_b[p]
OFF_PREB = OFF_GATEB + 1               # [p<64, 0] = pre_b[p]
OFF_HEADB = OFF_PREB + 1               # [p<2, 0] = head_b[p]
WBCOLS = OFF_HEADB + 1                 # 14


def pack_weights(inp):
    wm = np.zeros((128, WMCOLS), np.float32)
    pw = np.asarray(inp["proj_w"], np.float32)      # [3, 768, 128]
    wm[:, OFF_PROJ:OFF_W1] = (
        pw.reshape(NM, KIN, 128, 128).transpose(2, 0, 1, 3).reshape(128, -1)
    )
    w1 = np.asarray(inp["exp_w1"], np.float32)      # [8, 384, 128]
    wm[:, OFF_W1:OFF_W2] = (
        w1.reshape(NE, KX, 128, 128).transpose(2, 0, 1, 3).reshape(128, -1)
    )
    w2 = np.asarray(inp["exp_w2"], np.float32)      # [8, 128, 128]
    wm[:, OFF_W2:OFF_GATE] = w2.transpose(1, 0, 2).reshape(128, -1)
    gw = np.asarray(inp["gate_w"], np.float32)      # [384, 8]
    wm[:, OFF_GATE:OFF_PRE] = (
        gw.reshape(KX, 128, NE).transpose(1, 0, 2).reshape(128, -1)
    )
    wm[:, OFF_PRE:OFF_HEAD] = np.asarray(inp["pre_w"], np.float32)
    wm[:64, OFF_HEAD:OFF_B2] = np.asarray(inp["head_w"], np.float32)
    wm[:8, OFF_B2:OFF_B2 + 128] = np.asarray(inp["exp_b2"], np.float32)

    wb = np.zeros((128, WBCOLS), np.float32)
    wb[:, OFF_PROJB:OFF_B1] = np.asarray(inp["proj_b"], np.float32).T
    wb[:, OFF_B1:OFF_GATEB] = np.asarray(inp["exp_b1"], np.float32).T
    wb[:8, OFF_GATEB] = np.asarray(inp["gate_b"], np.float32)
    wb[:64, OFF_PREB] = np.asarray(inp["pre_b"], np.float32)
    wb[:2, OFF_HEADB] = np.asarray(inp["head_b"], np.float32)
    return wm, wb


def build_program(n_stripes=BL // STRIPE):
    """Build the per-core Bass program (identical on all cores)."""
    import concourse.bacc as bacc
    import concourse.bass as bass
    import concourse.bass_isa as bass_isa
    import concourse.mybir as mybir
    import concourse.tile as tile

    f32 = mybir.dt.float32
    f32r = mybir.dt.float32r
    AF = mybir.ActivationFunctionType
    bl = n_stripes * STRIPE

    nc = bacc.Bacc(
        "TRN2",
        target_bir_lowering=False,
        debug=False,
        enable_asserts=False,
    )

    featT = nc.dram_tensor("featT", [NM, D_IN, bl], f32r, kind="ExternalInput").ap()
    wmat = nc.dram_tensor("wmat", [128, WMCOLS], f32r, kind="ExternalInput").ap()
    wbias = nc.dram_tensor("wbias", [128, WBCOLS], f32, kind="ExternalInput").ap()
    outT = nc.dram_tensor("outT", [2, bl], f32, kind="ExternalOutput").ap()

    with tile.TileContext(nc) as tc, ExitStack() as ctx:
        wp_pool = ctx.enter_context(tc.tile_pool(name="wp", bufs=1))
        feat_pool = ctx.enter_context(tc.tile_pool(name="feat", bufs=4))
        grow_pool = ctx.enter_context(tc.tile_pool(name="grow", bufs=1))
        x_pool = ctx.enter_context(tc.tile_pool(name="x", bufs=6))
        gw_pool = ctx.enter_context(tc.tile_pool(name="gw", bufs=2))
        gb_pool = ctx.enter_context(tc.tile_pool(name="gb", bufs=2))
        h_pool = ctx.enter_context(tc.tile_pool(name="h", bufs=3))
        sh_pool = ctx.enter_context(tc.tile_pool(name="sh", bufs=3))
        f_pool = ctx.enter_context(tc.tile_pool(name="f", bufs=2))
        pen_pool = ctx.enter_context(tc.tile_pool(name="pen", bufs=2))
        o_pool = ctx.enter_context(tc.tile_pool(name="o", bufs=2))

        px_pool = ctx.enter_context(tc.tile_pool(name="px", bufs=2, space="PSUM"))
        ph_pool = ctx.enter_context(tc.tile_pool(name="ph", bufs=2, space="PSUM"))
        pf_pool = ctx.enter_context(tc.tile_pool(name="pf", bufs=2, space="PSUM"))
        ps_pool = ctx.enter_context(tc.tile_pool(name="ps", bufs=2, space="PSUM"))

        # preload packed weights once (f32r bits straight from DRAM) + biases
        W = wp_pool.tile([128, WMCOLS], f32r)
        nc.sync.dma_start(W[:], wmat[:])
        Bz = wp_pool.tile([128, WBCOLS], f32)
        nc.sync.dma_start(Bz[:], wbias[:])

        def wslice(off, n, parts=128):
            return W[:parts, off : off + n]

        def bslice(off, parts=128):
            return Bz[:parts, off : off + 1]

        featT_t = featT.rearrange("m (k p) b -> m p k b", p=128)

        for s in range(n_stripes):
            bsl = slice(s * STRIPE, (s + 1) * STRIPE)

            # ---- load features, cast f32 -> f32r (1.5 MB per modality) ----
            ft = []
            for m in range(NM):
                t = feat_pool.tile([128, KIN, STRIPE], f32r, tag="feat")
                nc.sync.dma_start(t[:], featT_t[m, :, :, bsl])
                ft.append(t)

            # ---- per-modality projection -> xT chunks ----
            xT = []
            for m in range(NM):
                px = px_pool.tile([128, STRIPE], f32, tag="px")
                for k in range(KIN):
                    nc.tensor.matmul(
                        px[:],
                        wslice(OFF_PROJ + m * KIN * 128 + k * 128, 128),
                        ft[m][:, k, :],
                        start=(k == 0),
                        stop=(k == KIN - 1),
                    )
                xt = x_pool.tile([128, STRIPE], f32r, tag="x")
                nc.scalar.activation(
                    xt[:], px[:], AF.Identity,
                    bias=bslice(OFF_PROJB + m), scale=1.0,
                )
                xT.append(xt)

            # ---- gate: softmax over 8 experts (partition axis) ----
            pg = ps_pool.tile([8, STRIPE], f32, tag="ps")
            for k in range(KX):
                nc.tensor.matmul(
                    pg[:],
                    wslice(OFF_GATE + k * NE, NE),
                    xT[k][:],
                    start=(k == 0),
                    stop=(k == KX - 1),
                )
            eT = gw_pool.tile([8, STRIPE], f32, tag="eT")
            nc.scalar.activation(
                eT[:], pg[:], AF.Exp, bias=bslice(OFF_GATEB, parts=8), scale=1.0
            )
            sT = gw_pool.tile([8, STRIPE], f32, tag="sT")
            nc.gpsimd.partition_all_reduce(
                sT[:], eT[:], channels=8, reduce_op=bass_isa.ReduceOp.add
            )
            rT = gw_pool.tile([8, STRIPE], f32, tag="rT")
            nc.vector.reciprocal_approx_fast(rT[:], sT[:])
            gwT = gw_pool.tile([8, STRIPE], f32r, tag="gwT")
            nc.vector.tensor_mul(gwT[:], eT[:], rT[:])

            # gather all 8 gate rows into one partition, broadcast to 128
            grow = grow_pool.tile([1, NE, STRIPE], f32r, tag="grow")
            nc.scalar.dma_start(grow[:], gwT[:])
            gball = gb_pool.tile([128, NE, STRIPE], f32r, tag="gb")
            nc.gpsimd.partition_broadcast(
                gball[:].rearrange("p e b -> p (e b)"),
                grow[:].rearrange("p e b -> p (e b)"),
                channels=128,
            )

            # ---- experts: h = relu(W1.T x + b1); sh = h * gw[e] ----
            sh = []
            for e in range(NE):
                ph = ph_pool.tile([128, STRIPE], f32, tag="ph")
                for k in range(KX):
                    nc.tensor.matmul(
                        ph[:],
                        wslice(OFF_W1 + e * KX * 128 + k * 128, 128),
                        xT[k][:],
                        start=(k == 0),
                        stop=(k == KX - 1),
                    )
                h = h_pool.tile([128, STRIPE], f32r, tag="h")
                nc.scalar.activation(
                    h[:], ph[:], AF.Relu, bias=bslice(OFF_B1 + e), scale=1.0
                )
                sht = sh_pool.tile([128, STRIPE], f32r, tag="sh")
                nc.vector.tensor_mul(sht[:], h[:], gball[:, e, :])
                sh.append(sht)

            # ---- fused = exp_b2.T @ gwT + sum_e W2[e].T @ sh[e] ----
            pf = pf_pool.tile([128, STRIPE], f32, tag="pf")
            nc.tensor.matmul(
                pf[:], wslice(OFF_B2, 128, parts=8), gwT[:],
                start=True, stop=False,
            )
            for e in range(NE):
                nc.tensor.matmul(
                    pf[:],
                    wslice(OFF_W2 + e * 128, 128),
                    sh[e][:],
                    start=False,
                    stop=(e == NE - 1),
                )
            fT = f_pool.tile([128, STRIPE], f32r, tag="f")
            nc.scalar.copy(fT[:], pf[:])

            # ---- penult = relu(pre_w.T @ fused + pre_b) ----
            pp = ps_pool.tile([64, STRIPE], f32, tag="ps")
            nc.tensor.matmul(pp[:], wslice(OFF_PRE, 64), fT[:],
                             start=True, stop=True)
            pen = pen_pool.tile([64, STRIPE], f32r, tag="pen")
            nc.scalar.activation(
                pen[:], pp[:], AF.Relu, bias=bslice(OFF_PREB, parts=64), scale=1.0
            )

            # ---- logits = head_w.T @ penult + head_b ----
            po = ps_pool.tile([2, STRIPE], f32, tag="ps")
            nc.tensor.matmul(po[:], wslice(OFF_HEAD, 2, parts=64), pen[:],
                             start=True, stop=True)
            ot = o_pool.tile([2, STRIPE], f32, tag="o")
            nc.scalar.activation(
                ot[:], po[:], AF.Identity, bias=bslice(OFF_HEADB, parts=2),
                scale=1.0,
            )
            nc.scalar.dma_start(outT[:, bsl], ot[:])

    nc.compile()
    return nc


_PROGRAM = None


def _get_program():
    global _PROGRAM
    if _PROGRAM is None:
        _PROGRAM = build_program()
    return _PROGRAM


def make_in_maps(inputs):
    """Host-side shard + layout prep: list of 8 per-core input maps."""
    wm, wb = pack_weights(inputs)
    feats = [
        np.asarray(inputs["feat_text"], np.float32),
        np.asarray(inputs["feat_audio"], np.float32),
        np.asarray(inputs["feat_video"], np.float32),
    ]
    in_maps = []
    for c in range(NCORES):
        sl = slice(c * BL, (c + 1) * BL)
        featT = np.stack([np.ascontiguousarray(f[sl].T) for f in feats])
        in_maps.append({"featT": featT, "wmat": wm, "wbias": wb})
    return in_maps


def run_on_hw(inputs, trace=False):
    from concourse.bass_utils import run_bass_kernel_spmd

    nc = _get_program()
    in_maps = make_in_maps(inputs)
    res = run_bass_kernel_spmd(
        nc, in_maps, core_ids=list(range(NCORES)), trace=trace
    )
    out = np.concatenate([r["outT"].T for r in res.results], axis=0)
    return out, res


def kernel(**inputs):
    out, _ = run_on_hw(inputs, trace=False)
    return out


# revision 19
# speedup vs baseline: 1.1337x; 1.1334x over previous
"""Trainium2 Bass kernel for nn_MoEFusion (multi-modal MoE fusion MLP).

Data-parallel across 8 NeuronCores: batch dim (32768) sharded into 8
slices of 4096, all weights (<1 MB) replicated. No collectives.

On-device dataflow (per core, feature-major "T" layout everywhere):
  featT [768, 4096] (host-pre-transposed) --DMA(cast f32r)--> SBUF
  projT[m] = proj_w[m].T @ featT[m]            (PE, f32r, accum K=768)
  xT = concat_m(projT + proj_b)                (ACT bias-add PSUM->SBUF)
  gateT = exp(gate_w.T @ xT + gate_b)          (PE + ACT Exp)
  gwT = gateT / colsum(gateT)                  (GPSIMD partition reduce, DVE)
  hT[e] = relu(W1[e].T @ xT + b1[e])           (PE + ACT Relu)
  shT[e] = hT[e] * bcast(gwT[e])               (GPSIMD bcast + DVE mul)
  fusedT = exp_b2.T @ gwT + sum_e W2[e].T @ shT[e]   (single PSUM accum)
  penT = relu(pre_w.T @ fusedT + pre_b)        (PE + ACT)
  outT = head_w.T @ penT + head_b              (PE + ACT) --DMA--> [2, 4096]
Host re-transposes/concats to [32768, 2].

float32r: PE streams fp32 data at full rate (1 col/cycle for N>=256);
walrus requires every f32r matmul operand to be *produced* as f32r, so
DMA loads cast f32->f32r (SWDGE) and ACT/DVE producers write f32r tiles.
"""

import sys

if "/opt/trn_rl_repo" not in sys.path:
    sys.path.insert(0, "/opt/trn_rl_repo")

from contextlib import ExitStack

import numpy as np

# ---- problem constants (hardcoded per contract) ----
B = 32768
NCORES = 8
BL = B // NCORES  # 4096 per core
STRIPE = 512
NM = 3
NE = 8
D_IN = 768
KIN = D_IN // 128  # 6
D_P = 128
D_X = 384
KX = D_X // 128  # 3

# ---- packed matmul-weight SBUF layout (columns of [128, WMCOLS], f32r) ----
OFF_PROJ = 0                           # [p, m*768 + k*128 + o] = proj_w[m, k*128+p, o]
OFF_W1 = OFF_PROJ + NM * KIN * 128     # 2304: [p, e*384 + k*128 + h] = exp_w1[e, k*128+p, h]
OFF_W2 = OFF_W1 + NE * KX * 128        # 5376: [p, e*128 + o] = exp_w2[e, p, o]
OFF_GATE = OFF_W2 + NE * 128           # 6400: [p, k*8 + e] = gate_w[k*128+p, e]
OFF_PRE = OFF_GATE + KX * NE           # 6424: [p, o] = pre_w[p, o]
OFF_HEAD = OFF_PRE + 64                # 6488: [p<64, c] = head_w[p, c]
OFF_B2 = OFF_HEAD + 2                  # 6490: [p<8, o] = exp_b2[p, o]
WMCOLS = OFF_B2 + 128                  # 6618

# ---- bias SBUF layout (columns of [128, WBCOLS], f32) ----
OFF_PROJB = 0                          # [p, m] = proj_b[m, p]
OFF_B1 = OFF_PROJB + NM                # [p, e] = exp_b1[e, p]
OFF_GATEB = OFF_B1 + NE                # [p<8, 0] = gate_b[p]
OFF_PREB = OFF_GATEB + 1               # [p<64, 0] = pre_b[p]
OFF_HEADB = OFF_PREB + 1               # [p<2, 0] = head_b[p]
WBCOLS = OFF_HEADB + 1                 # 14


def pack_weights(inp):
    wm = np.zeros((128, WMCOLS), np.float32)
    pw = np.asarray(inp["proj_w"], np.float32)      # [3, 768, 128]
    wm[:, OFF_PROJ:OFF_W1] = (
        pw.reshape(NM, KIN, 128, 128).transpose(2, 0, 1, 3).reshape(128, -1)
    )
    w1 = np.asarray(inp["exp_w1"], np.float32)      # [8, 384, 128]
    wm[:, OFF_W1:OFF_W2] = (
        w1.reshape(NE, KX, 128, 128).transpose(2, 0, 1, 3).reshape(128, -1)
    )
    w2 = np.asarray(inp["exp_w2"], np.float32)      # [8, 128, 128]
    wm[:, OFF_W2:OFF_GATE] = w2.transpose(1, 0, 2).reshape(128, -1)
    gw = np.asarray(inp["gate_w"], np.float32)      # [384, 8]
    wm[:, OFF_GATE:OFF_PRE] = (
        gw.reshape(KX, 128, NE).transpose(1, 0, 2).reshape(128, -1)
    )
    wm[:, OFF_PRE:OFF_HEAD] = np.asarray(inp["pre_w"], np.float32)
    wm[:64, OFF_HEAD:OFF_B2] = np.asarray(inp["head_w"], np.float32)
    wm[:8, OFF_B2:OFF_B2 + 128] = np.asarray(inp["exp_b2"], np.float32)

    wb = np.zeros((128, WBCOLS), np.float32)
    wb[:, OFF_PROJB:OFF_B1] = np.asarray(inp["proj_b"], np.float32).T
    wb[:, OFF_B1:OFF_GATEB] = np.asarray(inp["exp_b1"], np.float32).T
    wb[:8, OFF_GATEB] = np.asarray(inp["gate_b"], np.float32)
    wb[:64, OFF_PREB] = np.asarray(inp["pre_b"], np.float32)
    wb[:2, OFF_HEADB] = np.asarray(inp["head_b"], np.float32)
    return wm, wb


def build_program(n_stripes=BL // STRIPE):
    """Build the per-core Bass program (identical on all cores)."""
    import concourse.bacc as bacc
    import concourse.bass as bass
    import concourse.bass_isa as bass_isa
    import concourse.mybir as mybir
    import concourse.tile as tile

    f32 = mybir.dt.float32
    f32r = mybir.dt.float32r
    AF = mybir.ActivationFunctionType
    bl = n_stripes * STRIPE

    nc = bacc.Bacc(
        "TRN2",
        target_bir_lowering=False,
        debug=False,
        enable_asserts=False,
    )

    featT = nc.dram_tensor("featT", [NM, D_IN, bl], f32r, kind="ExternalInput").ap()
    wmat = nc.dram_tensor("wmat", [128, WMCOLS], f32r, kind="ExternalInput").ap()
    wbias = nc.dram_tensor("wbias", [128, WBCOLS], f32, kind="ExternalInput").ap()
    outT = nc.dram_tensor("outT", [2, bl], f32, kind="ExternalOutput").ap()

    with tile.TileContext(nc) as tc, ExitStack() as ctx:
        wp_pool = ctx.enter_context(tc.tile_pool(name="wp", bufs=1))
        feat_pool = ctx.enter_context(tc.tile_pool(name="feat", bufs=6))
        grow_pool = ctx.enter_context(tc.tile_pool(name="grow", bufs=1))
        x_pool = ctx.enter_context(tc.tile_pool(name="x", bufs=6))
        gw_pool = ctx.enter_context(tc.tile_pool(name="gw", bufs=2))
        gb_pool = ctx.enter_context(tc.tile_pool(name="gb", bufs=4))
        h_pool = ctx.enter_context(tc.tile_pool(name="h", bufs=3))
        sh_pool = ctx.enter_context(tc.tile_pool(name="sh", bufs=3))
        f_pool = ctx.enter_context(tc.tile_pool(name="f", bufs=2))
        pen_pool = ctx.enter_context(tc.tile_pool(name="pen", bufs=2))
        o_pool = ctx.enter_context(tc.tile_pool(name="o", bufs=2))

        px_pool = ctx.enter_context(tc.tile_pool(name="px", bufs=2, space="PSUM"))
        ph_pool = ctx.enter_context(tc.tile_pool(name="ph", bufs=2, space="PSUM"))
        pf_pool = ctx.enter_context(tc.tile_pool(name="pf", bufs=2, space="PSUM"))
        ps_pool = ctx.enter_context(tc.tile_pool(name="ps", bufs=2, space="PSUM"))

        # preload packed weights once (f32r bits straight from DRAM) + biases
        W = wp_pool.tile([128, WMCOLS], f32r)
        nc.sync.dma_start(W[:], wmat[:])
        Bz = wp_pool.tile([128, WBCOLS], f32)
        nc.sync.dma_start(Bz[:], wbias[:])

        def wslice(off, n, parts=128):
            return W[:parts, off : off + n]

        def bslice(off, parts=128):
            return Bz[:parts, off : off + 1]

        featT_t = featT.rearrange("m (k p) b -> m p k b", p=128)

        for s in range(n_stripes):
            bsl = slice(s * STRIPE, (s + 1) * STRIPE)

            # ---- load features, cast f32 -> f32r (1.5 MB per modality) ----
            ft = []
            for m in range(NM):
                t = feat_pool.tile([128, KIN, STRIPE], f32r, tag="feat")
                nc.sync.dma_start(t[:], featT_t[m, :, :, bsl])
                ft.append(t)

            # ---- per-modality projection -> xT chunks ----
            xT = []
            for m in range(NM):
                px = px_pool.tile([128, STRIPE], f32, tag="px")
                for k in range(KIN):
                    nc.tensor.matmul(
                        px[:],
                        wslice(OFF_PROJ + m * KIN * 128 + k * 128, 128),
                        ft[m][:, k, :],
                        start=(k == 0),
                        stop=(k == KIN - 1),
                    )
                xt = x_pool.tile([128, STRIPE], f32r, tag="x")
                nc.scalar.activation(
                    xt[:], px[:], AF.Identity,
                    bias=bslice(OFF_PROJB + m), scale=1.0,
                )
                xT.append(xt)

            # ---- gate: softmax over 8 experts (partition axis) ----
            pg = ps_pool.tile([8, STRIPE], f32, tag="ps")
            for k in range(KX):
                nc.tensor.matmul(
                    pg[:],
                    wslice(OFF_GATE + k * NE, NE),
                    xT[k][:],
                    start=(k == 0),
                    stop=(k == KX - 1),
                )
            eT = gw_pool.tile([8, STRIPE], f32, tag="eT")
            nc.scalar.activation(
                eT[:], pg[:], AF.Exp, bias=bslice(OFF_GATEB, parts=8), scale=1.0
            )
            sT = gw_pool.tile([8, STRIPE], f32, tag="sT")
            nc.gpsimd.partition_all_reduce(
                sT[:], eT[:], channels=8, reduce_op=bass_isa.ReduceOp.add
            )
            rT = gw_pool.tile([8, STRIPE], f32, tag="rT")
            nc.vector.reciprocal_approx_fast(rT[:], sT[:])
            gwT = gw_pool.tile([8, STRIPE], f32r, tag="gwT")
            nc.vector.tensor_mul(gwT[:], eT[:], rT[:])

            # gather all 8 gate rows into one partition (single tiny DMA)
            grow = grow_pool.tile([1, NE, STRIPE], f32r, tag="grow")
            nc.scalar.dma_start(grow[:], gwT[:])

            # ---- experts: h = relu(W1.T x + b1); sh = h * gw[e] ----
            sh = []
            for e in range(NE):
                ph = ph_pool.tile([128, STRIPE], f32, tag="ph")
                for k in range(KX):
                    nc.tensor.matmul(
                        ph[:],
                        wslice(OFF_W1 + e * KX * 128 + k * 128, 128),
                        xT[k][:],
                        start=(k == 0),
                        stop=(k == KX - 1),
                    )
                h = h_pool.tile([128, STRIPE], f32r, tag="h")
                nc.scalar.activation(
                    h[:], ph[:], AF.Relu, bias=bslice(OFF_B1 + e), scale=1.0
                )
                gb = gb_pool.tile([128, STRIPE], f32r, tag="gb")
                nc.gpsimd.partition_broadcast(gb[:], grow[0:1, e, :], channels=128)
                sht = sh_pool.tile([128, STRIPE], f32r, tag="sh")
                nc.vector.tensor_mul(sht[:], h[:], gb[:])
                sh.append(sht)

            # ---- fused = exp_b2.T @ gwT + sum_e W2[e].T @ sh[e] ----
            pf = pf_pool.tile([128, STRIPE], f32, tag="pf")
            nc.tensor.matmul(
                pf[:], wslice(OFF_B2, 128, parts=8), gwT[:],
                start=True, stop=False,
            )
            for e in range(NE):
                nc.tensor.matmul(
                    pf[:],
                    wslice(OFF_W2 + e * 128, 128),
                    sh[e][:],
                    start=False,
                    stop=(e == NE - 1),
                )
            fT = f_pool.tile([128, STRIPE], f32r, tag="f")
            nc.scalar.copy(fT[:], pf[:])

            # ---- penult = relu(pre_w.T @ fused + pre_b) ----
            pp = ps_pool.tile([64, STRIPE], f32, tag="ps")
            nc.tensor.matmul(pp[:], wslice(OFF_PRE, 64), fT[:],
                             start=True, stop=True)
            pen = pen_pool.tile([64, STRIPE], f32r, tag="pen")
            nc.scalar.activation(
                pen[:], pp[:], AF.Relu, bias=bslice(OFF_PREB, parts=64), scale=1.0
            )

            # ---- logits = head_w.T @ penult + head_b ----
            po = ps_pool.tile([2, STRIPE], f32, tag="ps")
            nc.tensor.matmul(po[:], wslice(OFF_HEAD, 2, parts=64), pen[:],
                             start=True, stop=True)
            ot = o_pool.tile([2, STRIPE], f32, tag="o")
            nc.scalar.activation(
                ot[:], po[:], AF.Identity, bias=bslice(OFF_HEADB, parts=2),
                scale=1.0,
            )
            nc.scalar.dma_start(outT[:, bsl], ot[:])

    nc.compile()
    return nc


_PROGRAM = None


def _get_program():
    global _PROGRAM
    if _PROGRAM is None:
        _PROGRAM = build_program()
    return _PROGRAM


def make_in_maps(inputs):
    """Host-side shard + layout prep: list of 8 per-core input maps."""
    wm, wb = pack_weights(inputs)
    feats = [
        np.asarray(inputs["feat_text"], np.float32),
        np.asarray(inputs["feat_audio"], np.float32),
        np.asarray(inputs["feat_video"], np.float32),
    ]
    in_maps = []
    for c in range(NCORES):
        sl = slice(c * BL, (c + 1) * BL)
        featT = np.stack([np.ascontiguousarray(f[sl].T) for f in feats])
        in_maps.append({"featT": featT, "wmat": wm, "wbias": wb})
    return in_maps


def run_on_hw(inputs, trace=False):
    from concourse.bass_utils import run_bass_kernel_spmd

    nc = _get_program()
    in_maps = make_in_maps(inputs)
    res = run_bass_kernel_spmd(
        nc, in_maps, core_ids=list(range(NCORES)), trace=trace
    )
    out = np.concatenate([r["outT"].T for r in res.results], axis=0)
    return out, res


def kernel(**inputs):
    out, _ = run_on_hw(inputs, trace=False)
    return out


# revision 23
# speedup vs baseline: 1.3079x; 1.1536x over previous
"""Trainium2 Bass kernel for nn_MoEFusion (multi-modal MoE fusion MLP).

Data-parallel across 8 NeuronCores: batch dim (32768) sharded into 8
slices of 4096, all weights (<1 MB) replicated. No collectives.

On-device dataflow (per core, feature-major "T" layout everywhere):
  featT [768, 4096] (host-pre-transposed) --DMA(cast f32r)--> SBUF
  projT[m] = proj_w[m].T @ featT[m]            (PE, f32r, accum K=768)
  xT = concat_m(projT + proj_b)                (ACT bias-add PSUM->SBUF)
  gateT = exp(gate_w.T @ xT + gate_b)          (PE + ACT Exp)
  gwT = gateT / colsum(gateT)                  (GPSIMD partition reduce, DVE)
  hT[e] = relu(W1[e].T @ xT + b1[e])           (PE + ACT Relu)
  shT[e] = hT[e] * bcast(gwT[e])               (GPSIMD bcast + DVE mul)
  fusedT = exp_b2.T @ gwT + sum_e W2[e].T @ shT[e]   (single PSUM accum)
  penT = relu(pre_w.T @ fusedT + pre_b)        (PE + ACT)
  outT = head_w.T @ penT + head_b              (PE + ACT) --DMA--> [2, 4096]
Host re-transposes/concats to [32768, 2].

float32r: PE streams fp32 data at full rate (1 col/cycle for N>=256);
walrus requires every f32r matmul operand to be *produced* as f32r, so
DMA loads cast f32->f32r (SWDGE) and ACT/DVE producers write f32r tiles.
"""

import sys

if "/opt/trn_rl_repo" not in sys.path:
    sys.path.insert(0, "/opt/trn_rl_repo")

from contextlib import ExitStack

import numpy as np

# ---- problem constants (hardcoded per contract) ----
B = 32768
NCORES = 8
BL = B // NCORES  # 4096 per core
STRIPE = 512
NM = 3
NE = 8
D_IN = 768
KIN = D_IN // 128  # 6
D_P = 128
D_X = 384
KX = D_X // 128  # 3

# ---- packed matmul-weight SBUF layout (columns of [128, WMCOLS], f32r) ----
OFF_PROJ = 0                           # [p, m*768 + k*128 + o] = proj_w[m, k*128+p, o]
OFF_W1 = OFF_PROJ + NM * KIN * 128     # 2304: [p, e*384 + k*128 + h] = exp_w1[e, k*128+p, h]
OFF_W2 = OFF_W1 + NE * KX * 128        # 5376: [p, e*128 + o] = exp_w2[e, p, o]
OFF_GATE = OFF_W2 + NE * 128           # 6400: [p, k*8 + e] = gate_w[k*128+p, e]
OFF_PRE = OFF_GATE + KX * NE           # 6424: [p, o] = pre_w[p, o]
OFF_HEAD = OFF_PRE + 64                # 6488: [p<64, c] = head_w[p, c]
OFF_B2 = OFF_HEAD + 2                  # 6490: [p<8, o] = exp_b2[p, o]
WMCOLS = OFF_B2 + 128                  # 6618

# ---- bias SBUF layout (columns of [128, WBCOLS], f32) ----
OFF_PROJB = 0                          # [p, m] = proj_b[m, p]
OFF_B1 = OFF_PROJB + NM                # [p, e] = exp_b1[e, p]
OFF_GATEB = OFF_B1 + NE                # [p<8, 0] = gate_b[p]
OFF_PREB = OFF_GATEB + 1               # [p<64, 0] = pre_b[p]
OFF_HEADB = OFF_PREB + 1               # [p<2, 0] = head_b[p]
WBCOLS = OFF_HEADB + 1                 # 14


def pack_weights(inp):
    wm = np.zeros((128, WMCOLS), np.float32)
    pw = np.asarray(inp["proj_w"], np.float32)      # [3, 768, 128]
    wm[:, OFF_PROJ:OFF_W1] = (
        pw.reshape(NM, KIN, 128, 128).transpose(2, 0, 1, 3).reshape(128, -1)
    )
    w1 = np.asarray(inp["exp_w1"], np.float32)      # [8, 384, 128]
    wm[:, OFF_W1:OFF_W2] = (
        w1.reshape(NE, KX, 128, 128).transpose(2, 0, 1, 3).reshape(128, -1)
    )
    w2 = np.asarray(inp["exp_w2"], np.float32)      # [8, 128, 128]
    wm[:, OFF_W2:OFF_GATE] = w2.transpose(1, 0, 2).reshape(128, -1)
    gw = np.asarray(inp["gate_w"], np.float32)      # [384, 8]
    wm[:, OFF_GATE:OFF_PRE] = (
        gw.reshape(KX, 128, NE).transpose(1, 0, 2).reshape(128, -1)
    )
    wm[:, OFF_PRE:OFF_HEAD] = np.asarray(inp["pre_w"], np.float32)
    wm[:64, OFF_HEAD:OFF_B2] = np.asarray(inp["head_w"], np.float32)
    wm[:8, OFF_B2:OFF_B2 + 128] = np.asarray(inp["exp_b2"], np.float32)

    wb = np.zeros((128, WBCOLS), np.float32)
    wb[:, OFF_PROJB:OFF_B1] = np.asarray(inp["proj_b"], np.float32).T
    wb[:, OFF_B1:OFF_GATEB] = np.asarray(inp["exp_b1"], np.float32).T
    wb[:8, OFF_GATEB] = np.asarray(inp["gate_b"], np.float32)
    wb[:64, OFF_PREB] = np.asarray(inp["pre_b"], np.float32)
    wb[:2, OFF_HEADB] = np.asarray(inp["head_b"], np.float32)
    return wm, wb


def build_program(n_stripes=BL // STRIPE):
    """Build the per-core Bass program (identical on all cores)."""
    import concourse.bacc as bacc
    import concourse.bass as bass
    import concourse.bass_isa as bass_isa
    import concourse.mybir as mybir
    import concourse.tile as tile

    f32 = mybir.dt.float32
    f32r = mybir.dt.float32r
    AF = mybir.ActivationFunctionType
    bl = n_stripes * STRIPE

    nc = bacc.Bacc(
        "TRN2",
        target_bir_lowering=False,
        debug=False,
        enable_asserts=False,
    )

    featT = nc.dram_tensor("featT", [NM, D_IN, bl], f32r, kind="ExternalInput").ap()
    wmat = nc.dram_tensor("wmat", [128, WMCOLS], f32r, kind="ExternalInput").ap()
    wbias = nc.dram_tensor("wbias", [128, WBCOLS], f32, kind="ExternalInput").ap()
    outT = nc.dram_tensor("outT", [2, bl], f32, kind="ExternalOutput").ap()

    with tile.TileContext(nc) as tc, ExitStack() as ctx:
        wp_pool = ctx.enter_context(tc.tile_pool(name="wp", bufs=1))
        feat_pool = ctx.enter_context(tc.tile_pool(name="feat", bufs=5))
        grow_pool = ctx.enter_context(tc.tile_pool(name="grow", bufs=1))
        x_pool = ctx.enter_context(tc.tile_pool(name="x", bufs=6))
        gw_pool = ctx.enter_context(tc.tile_pool(name="gw", bufs=2))
        gb_pool = ctx.enter_context(tc.tile_pool(name="gb", bufs=4))
        h_pool = ctx.enter_context(tc.tile_pool(name="h", bufs=3))
        sh_pool = ctx.enter_context(tc.tile_pool(name="sh", bufs=11))
        f_pool = ctx.enter_context(tc.tile_pool(name="f", bufs=2))
        pen_pool = ctx.enter_context(tc.tile_pool(name="pen", bufs=2))
        o_pool = ctx.enter_context(tc.tile_pool(name="o", bufs=2))

        px_pool = ctx.enter_context(tc.tile_pool(name="px", bufs=2, space="PSUM"))
        ph_pool = ctx.enter_context(tc.tile_pool(name="ph", bufs=2, space="PSUM"))
        pf_pool = ctx.enter_context(tc.tile_pool(name="pf", bufs=2, space="PSUM"))
        ps_pool = ctx.enter_context(tc.tile_pool(name="ps", bufs=2, space="PSUM"))

        # preload packed weights once (f32r bits straight from DRAM) + biases
        W = wp_pool.tile([128, WMCOLS], f32r)
        nc.sync.dma_start(W[:], wmat[:])
        Bz = wp_pool.tile([128, WBCOLS], f32)
        nc.sync.dma_start(Bz[:], wbias[:])

        def wslice(off, n, parts=128):
            return W[:parts, off : off + n]

        def bslice(off, parts=128):
            return Bz[:parts, off : off + 1]

        featT_t = featT.rearrange("m (k p) b -> m p k b", p=128)

        # Software pipeline: stage 2 (l2 accumulation + pre/head) for stripe
        # s-1 is emitted after stage 1 of stripe s, so the PE instruction
        # stream never stalls on the gate-softmax/broadcast chain.
        pend = None  # (sh, gwT, bsl) of previous stripe

        def emit_tail(pend):
            sh, gwT, bsl = pend
            # ---- fused = exp_b2.T @ gwT + sum_e W2[e].T @ sh[e] ----
            pf = pf_pool.tile([128, STRIPE], f32, tag="pf")
            nc.tensor.matmul(
                pf[:], wslice(OFF_B2, 128, parts=8), gwT[:],
                start=True, stop=False,
            )
            for e in range(NE):
                nc.tensor.matmul(
                    pf[:],
                    wslice(OFF_W2 + e * 128, 128),
                    sh[e][:],
                    start=False,
                    stop=(e == NE - 1),
                )
            fT = f_pool.tile([128, STRIPE], f32r, tag="f")
            nc.scalar.copy(fT[:], pf[:])

            # ---- penult = relu(pre_w.T @ fused + pre_b) ----
            pp = ps_pool.tile([64, STRIPE], f32, tag="ps")
            nc.tensor.matmul(pp[:], wslice(OFF_PRE, 64), fT[:],
                             start=True, stop=True)
            pen = pen_pool.tile([64, STRIPE], f32r, tag="pen")
            nc.scalar.activation(
                pen[:], pp[:], AF.Relu, bias=bslice(OFF_PREB, parts=64), scale=1.0
            )

            # ---- logits = head_w.T @ penult + head_b ----
            po = ps_pool.tile([2, STRIPE], f32, tag="ps")
            nc.tensor.matmul(po[:], wslice(OFF_HEAD, 2, parts=64), pen[:],
                             start=True, stop=True)
            ot = o_pool.tile([2, STRIPE], f32, tag="o")
            nc.scalar.activation(
                ot[:], po[:], AF.Identity, bias=bslice(OFF_HEADB, parts=2),
                scale=1.0,
            )
            nc.scalar.dma_start(outT[:, bsl], ot[:])

        for s in range(n_stripes):
            bsl = slice(s * STRIPE, (s + 1) * STRIPE)

            # ---- load features (1.5 MB per modality) ----
            ft = []
            for m in range(NM):
                t = feat_pool.tile([128, KIN, STRIPE], f32r, tag="feat")
                nc.sync.dma_start(t[:], featT_t[m, :, :, bsl])
                ft.append(t)

            # ---- per-modality projection -> xT chunks ----
            xT = []
            for m in range(NM):
                px = px_pool.tile([128, STRIPE], f32, tag="px")
                for k in range(KIN):
                    nc.tensor.matmul(
                        px[:],
                        wslice(OFF_PROJ + m * KIN * 128 + k * 128, 128),
                        ft[m][:, k, :],
                        start=(k == 0),
                        stop=(k == KIN - 1),
                    )
                xt = x_pool.tile([128, STRIPE], f32r, tag="x")
                nc.scalar.activation(
                    xt[:], px[:], AF.Identity,
                    bias=bslice(OFF_PROJB + m), scale=1.0,
                )
                xT.append(xt)

            # ---- gate: softmax over 8 experts (partition axis) ----
            pg = ps_pool.tile([8, STRIPE], f32, tag="ps")
            for k in range(KX):
                nc.tensor.matmul(
                    pg[:],
                    wslice(OFF_GATE + k * NE, NE),
                    xT[k][:],
                    start=(k == 0),
                    stop=(k == KX - 1),
                )
            eT = gw_pool.tile([8, STRIPE], f32, tag="eT")
            nc.scalar.activation(
                eT[:], pg[:], AF.Exp, bias=bslice(OFF_GATEB, parts=8), scale=1.0
            )
            sT = gw_pool.tile([8, STRIPE], f32, tag="sT")
            nc.gpsimd.partition_all_reduce(
                sT[:], eT[:], channels=8, reduce_op=bass_isa.ReduceOp.add
            )
            rT = gw_pool.tile([8, STRIPE], f32, tag="rT")
            nc.vector.reciprocal_approx_fast(rT[:], sT[:])
            gwT = gw_pool.tile([8, STRIPE], f32r, tag="gwT")
            nc.vector.tensor_mul(gwT[:], eT[:], rT[:])

            # gather all 8 gate rows into one partition (single tiny DMA)
            grow = grow_pool.tile([1, NE, STRIPE], f32r, tag="grow")
            nc.scalar.dma_start(grow[:], gwT[:])

            # ---- experts: h = relu(W1.T x + b1); sh = h * gw[e] ----
            sh = []
            for e in range(NE):
                ph = ph_pool.tile([128, STRIPE], f32, tag="ph")
                for k in range(KX):
                    nc.tensor.matmul(
                        ph[:],
                        wslice(OFF_W1 + e * KX * 128 + k * 128, 128),
                        xT[k][:],
                        start=(k == 0),
                        stop=(k == KX - 1),
                    )
                h = h_pool.tile([128, STRIPE], f32r, tag="h")
                nc.scalar.activation(
                    h[:], ph[:], AF.Relu, bias=bslice(OFF_B1 + e), scale=1.0
                )
                gb = gb_pool.tile([128, STRIPE], f32r, tag="gb")
                nc.gpsimd.partition_broadcast(gb[:], grow[0:1, e, :], channels=128)
                sht = sh_pool.tile([128, STRIPE], f32r, tag="sh")
                nc.vector.tensor_mul(sht[:], h[:], gb[:])
                sh.append(sht)

            if pend is not None:
                emit_tail(pend)
            pend = (sh, gwT, bsl)

        emit_tail(pend)

    nc.compile()
    return nc


_PROGRAM = None


def _get_program():
    global _PROGRAM
    if _PROGRAM is None:
        _PROGRAM = build_program()
    return _PROGRAM


def make_in_maps(inputs):
    """Host-side shard + layout prep: list of 8 per-core input maps."""
    wm, wb = pack_weights(inputs)
    feats = [
        np.asarray(inputs["feat_text"], np.float32),
        np.asarray(inputs["feat_audio"], np.float32),
        np.asarray(inputs["feat_video"], np.float32),
    ]
    in_maps = []
    for c in range(NCORES):
        sl = slice(c * BL, (c + 1) * BL)
        featT = np.stack([np.ascontiguousarray(f[sl].T) for f in feats])
        in_maps.append({"featT": featT, "wmat": wm, "wbias": wb})
    return in_maps


def run_on_hw(inputs, trace=False):
    from concourse.bass_utils import run_bass_kernel_spmd

    nc = _get_program()
    in_maps = make_in_maps(inputs)
    res = run_bass_kernel_spmd(
        nc, in_maps, core_ids=list(range(NCORES)), trace=trace
    )
    out = np.concatenate([r["outT"].T for r in res.results], axis=0)
    return out, res


def kernel(**inputs):
    out, _ = run_on_hw(inputs, trace=False)
    return out
